# revision 33
# baseline (speedup 1.0000x reference)
"""Trainium2 Bass kernel for teacher-forced GRU decoder (nn_DecoderRNN).

Problem (hardcoded from spec):
  B=2048, T=160, H=512, EMB=64, V=128, SOS=0
  reference computes, per step t (tokens teacher-forced, x = relu(emb[tok])):
      gi = x @ W_ih.T + b_ih ; gh = h @ W_hh.T + b_hh
      r = sig(gi_r + gh_r); z = sig(gi_z + gh_z); n = tanh(gi_n + r*gh_n)
      h = (1-z)*n + z*h ; logits = h @ W_out.T + b_out
  outputs: log_softmax(logits) [T, B, V] and final hidden [1, B, H].

Strategy (8 cores, data-parallel over batch, 256 rows/core):
  - Everything transposed on-chip: hidden state hT stored [128 part, 4, 256]
    (partition = H row within chunk, free = batch), so the recurrent matmul
    needs no per-step transposes and gates are elementwise in this layout.
  - Since relu(embedding) has only V=128 distinct rows, precompute (on device,
    fp32) a table giT[v, 3H] = relu(emb)[v] @ W_ih.T + b_ih (+ b_hh folded in
    for the r/z parts).  Per step, gi rows are "gathered" via a one-hot
    matmul that accumulates directly into PSUM with the W_hh matmuls.
  - Per-step matmuls in float32r (full PE speed, ~1e-4 rel err).
  - Raw logits stored to HBM per step; log_softmax applied in a final phase
    (avoids thrashing ACT tables between sigmoid/tanh and exp/ln sets).
"""
import sys

sys.path.insert(0, "/opt/trn_rl_repo")

import numpy as np

import concourse.bass as bass
import concourse.mybir as mybir
import concourse.tile as tile
from concourse import bacc
from concourse.bass_utils import run_bass_kernel_spmd

F32 = mybir.dt.float32
AF = mybir.ActivationFunctionType
OP = mybir.AluOpType

B = 2048
T = 160
H = 512
EMB = 64
V = 128
NCORES = 8
BL = B // NCORES          # 256 batch rows per core
KC = H // 128             # 4 hidden chunks
NC3 = 3 * H // 128        # 12 output chunks of 3H

LAST_EXEC_NS = None
_CACHE = {}


def build(T_steps=T, mm_dtype=mybir.dt.float32r, trace=False, dbg=False, skel=0):
    DT = mm_dtype
    nc = bacc.Bacc("TRN2", target_bir_lowering=False, debug=False)
    dbg_outs = {}
    if dbg:
        for nm in ("d_r", "d_z", "d_u", "d_v", "d_s", "d_g"):
            dbg_outs[nm] = nc.dram_tensor(nm, [128, KC, BL], F32, kind="ExternalOutput")
        dbg_outs["d_giT"] = nc.dram_tensor("d_giT", [128, NC3, 128], F32, kind="ExternalOutput")
        dbg_outs["d_h0"] = nc.dram_tensor("d_h0", [128, KC, BL], F32, kind="ExternalOutput")

    # ---------------- I/O ----------------
    h0t = nc.dram_tensor("h0t", [128, KC, BL], DT, kind="ExternalInput")
    onehot = nc.dram_tensor("onehot", [T_steps, 128, BL], DT, kind="ExternalInput")
    whht = nc.dram_tensor("whht", [128, KC, NC3, 128], DT, kind="ExternalInput")
    embt = nc.dram_tensor("embt", [EMB, V], F32, kind="ExternalInput")
    wih_aug = nc.dram_tensor("wih_aug", [128, 3 * H], F32, kind="ExternalInput")
    bhhn = nc.dram_tensor("bhhn", [128, KC], F32, kind="ExternalInput")
    woutt = nc.dram_tensor("woutt", [128, KC, V], DT, kind="ExternalInput")
    bout = nc.dram_tensor("bout", [1, V], DT, kind="ExternalInput")

    logp = nc.dram_tensor("logp", [T_steps, BL, V], F32, kind="ExternalOutput")
    hlast = nc.dram_tensor("hlast", [128, KC, BL], DT, kind="ExternalOutput")

    with tile.TileContext(nc) as tc:
        with (
            tc.tile_pool(name="consts", bufs=1) as consts,
            tc.tile_pool(name="hpool", bufs=3) as hpool,
            tc.tile_pool(name="gates", bufs=2) as gates,
            tc.tile_pool(name="ohp", bufs=4) as ohp,
            tc.tile_pool(name="lsb", bufs=3) as lsbp,
            tc.tile_pool(name="pA", bufs=1, space="PSUM") as pA,
            tc.tile_pool(name="pB", bufs=1, space="PSUM") as pB,
            tc.tile_pool(name="dram", bufs=1, space="DRAM") as dramp,
            tc.tile_pool(name="fin", bufs=4) as fin,
        ):
            # ------------- load constants -------------
            whht_sb = consts.tile([128, KC, NC3, 128], DT)
            nc.sync.dma_start(whht_sb[:], whht[:])
            woutt_sb = consts.tile([128, KC, V], DT)
            nc.sync.dma_start(woutt_sb[:], woutt[:])
            bout_sb = consts.tile([1, V], DT)
            nc.sync.dma_start(bout_sb[:], bout[:])
            bhhn_sb = consts.tile([128, KC], F32)
            nc.sync.dma_start(bhhn_sb[:], bhhn[:])
            wih_sb = consts.tile([128, 3 * H], F32)
            nc.sync.dma_start(wih_sb[:], wih_aug[:])
            embt_sb = consts.tile([EMB, V], F32)
            nc.sync.dma_start(embt_sb[:], embt[:])

            ones_f = consts.tile([1, BL], F32)
            nc.vector.memset(ones_f[:], 1.0)
            ones_sb = consts.tile([1, BL], DT)
            nc.vector.tensor_copy(ones_sb[:], ones_f[:])
            ones128_sb = consts.tile([1, 128], DT)
            nc.vector.tensor_copy(ones128_sb[:], ones_f[:, :128])

            # ------------- giT table (one-time, fp32 exact) -------------
            # lhsT_aug[k, v] = relu(emb)[v, k] for k<EMB ; 1.0 at k=EMB ; 0 else
            lhsT_aug = consts.tile([128, V], F32)
            nc.vector.memset(lhsT_aug[:], 0.0)
            nc.vector.tensor_scalar_max(lhsT_aug[:EMB, :], embt_sb[:], 0.0)
            nc.vector.memset(lhsT_aug[EMB : EMB + 1, :], 1.0)

            giT = consts.tile([128, NC3, 128], DT)  # [v, n-chunk, n-in-chunk]
            for nn in range(3):
                pg = pA.tile([128, 12, BL], F32, tag="A")
                nc.tensor.matmul(
                    pg[:, 0:2, :].rearrange("p a b -> p (a b)"),
                    lhsT_aug[:],
                    wih_sb[:, nn * 512 : (nn + 1) * 512],
                    start=True,
                    stop=True,
                )
                nc.scalar.copy(
                    giT[:, nn * 4 : (nn + 1) * 4, :].rearrange("p a b -> p (a b)"),
                    pg[:, 0:2, :].rearrange("p a b -> p (a b)"),
                )

            bout_bc = consts.tile([128, 2, V], F32)
            pgb = pB.tile([128, 4, BL], F32, tag="B", name="pgb")
            nc.tensor.matmul(
                pgb[:, 0, 0:V], ones128_sb[:], bout_sb[:], start=True, stop=True
            )
            nc.scalar.copy(bout_bc[:, 0, :], pgb[:, 0, 0:V])
            nc.scalar.copy(bout_bc[:, 1, :], pgb[:, 0, 0:V])

            def giT_l(n_idx):
                return giT[:, n_idx, :]

            # ------------- initial hidden -------------
            hT = hpool.tile([128, KC, BL], DT, tag="h")
            nc.sync.dma_start(hT[:], h0t[:])
            if dbg:
                nc.gpsimd.dma_start(dbg_outs["d_giT"].ap()[:], giT[:].bitcast(F32))
                nc.gpsimd.dma_start(dbg_outs["d_h0"].ap()[:], hT[:].bitcast(F32))

            logits_dram = dramp.tile([T_steps, BL, V], F32)

            # logits for step t are emitted one iteration later (inside the
            # k01/k23 MM stream of step t+1) so the in-order PE queue never
            # waits on the freshly-produced h of the same step.
            def emit_logits(h_tile, t_out, ps_area):
                # logits staged in slice 0 of the step's A-tile (bank 0);
                # copied out before the rz matmuls reclaim the bank.
                hnr = h_tile[:]
                for bc in range(2):
                    sl = ps_area[:, 0, bc * 128 : (bc + 1) * 128]
                    for k in range(KC):
                        nc.tensor.matmul(
                            sl,
                            hnr[:, k, bc * 128 : (bc + 1) * 128],
                            woutt_sb[:, k, :],
                            start=(k == 0 and bc == 0),
                            stop=(k == KC - 1),
                        )

                logit_sb = lsbp.tile([128, 2, V], F32, tag="lg")
                nc.vector.tensor_tensor(
                    logit_sb[:],
                    ps_area[:, 0, :].rearrange("p (c v) -> p c v", c=2),
                    bout_bc[:],
                    OP.add,
                )
                nc.sync.dma_start(
                    logits_dram[t_out].rearrange("(c p) v -> p c v", p=128),
                    logit_sb[:],
                )

            # ------------- recurrence -------------
            # Gate math at half granularity (H chunks 01 / 23).  Matmuls are
            # emitted k-chunks {0,1} first, then {2,3}, so the in-order PE
            # queue can start step t as soon as h_new chunks 0-1 of step t-1
            # exist (chunks 2-3 land ~a chain-latency later).
            for t in range(T_steps):
                oh = ohp.tile([128, BL], DT, tag="oh")
                nc.sync.dma_start(oh[:], onehot[t])

                hr = hT[:]

                psA = pA.tile([128, 12, BL], F32, tag="A", name=f"A{t}")
                psB = pB.tile([128, 4, BL], F32, tag="B", name=f"B{t}")

                # PSUM protocol: one start=True per bank (first touch of
                # the step); bank-mates rely on per-element has_written
                # overwrite-then-accumulate.  A-tile slices: r->0-3, z->4-7,
                # h_n->8-11 (logits of the previous step stage in slice 0
                # first and are copied out before rz reclaims bank 0).
                # B-tile slices: i_n->0-3.  Long sweeps, few tiles: this is
                # the fast PE stream shape on hardware.
                def mm_w(sl_ap, n_idx, ks, start_k0):
                    for k in ks:
                        nc.tensor.matmul(
                            sl_ap,
                            whht_sb[:, k, n_idx, :],
                            hr[:, k, :],
                            start=(start_k0 and k == ks[0]),
                            stop=False,
                        )

                # logits of the previous step: head of the A-tile stream
                if t > 0:
                    emit_logits(hT, t - 1, psA)

                # ---- k01 block
                for c in range(KC):          # a_r
                    nc.tensor.matmul(psA[:, c, :], giT_l(c), oh[:],
                                     start=(c % 2 == 0), stop=False)
                    mm_w(psA[:, c, :], c, (0, 1), False)
                for c in range(KC):          # a_z
                    nc.tensor.matmul(psA[:, 4 + c, :], giT_l(4 + c), oh[:],
                                     start=(c % 2 == 0), stop=False)
                    mm_w(psA[:, 4 + c, :], 4 + c, (0, 1), False)
                for c in range(KC):          # gh_n
                    mm_w(psA[:, 8 + c, :], 8 + c, (0, 1), c % 2 == 0)
                for c in range(KC):          # i_n
                    # NB: start+stop on a single matmul serializes the PE
                    # stream (~10x per-MM cost on HW); leave the group open.
                    nc.tensor.matmul(psB[:, c, :], giT_l(8 + c), oh[:],
                                     start=(c % 2 == 0), stop=False)

                # ---- k23 block
                for c in range(KC):
                    mm_w(psA[:, c, :], c, (2, 3), False)
                for c in range(KC):
                    mm_w(psA[:, 4 + c, :], 4 + c, (2, 3), False)
                for c in range(KC):
                    mm_w(psA[:, 8 + c, :], 8 + c, (2, 3), False)

                r_sb = gates.tile([128, KC, BL], F32, tag="r")
                z_sb = gates.tile([128, KC, BL], F32, tag="z")
                hT_new = hpool.tile([128, KC, BL], DT, tag="h")

                # State is reparametrized as g = h + 1, which removes every
                # additive constant from the gate algebra:
                #   s = sigmoid(2v)  (= (tanh(v)+1)/2)
                #   g_new = z*g + (1-z)*(2s-1) + 1 - z + z = z*g - 2*(z-1)*s
                # The W*g vs W*h discrepancy (rowsum(W)) is folded into the
                # bias rows host-side.  ACT runs ONLY Sigmoid all loop long,
                # so the activation table set is loaded exactly once.
                for hf in range(2):  # half = chunks [2*hf, 2*hf+1]
                    c0, c1 = 2 * hf, 2 * hf + 2

                    rh = r_sb[:, c0:c1, :]
                    nc.scalar.activation(rh, psA[:, c0:c1, :], AF.Sigmoid)
                    zh = z_sb[:, c0:c1, :]
                    nc.scalar.activation(zh, psA[:, 4 + c0 : 4 + c1, :], AF.Sigmoid)

                    u_sb = gates.tile([128, 2, BL], F32, tag="u")
                    for c in (c0, c0 + 1):
                        nc.vector.scalar_tensor_tensor(
                            u_sb[:, c - c0, :],
                            psA[:, 8 + c, :],
                            bhhn_sb[:, c : c + 1],
                            r_sb[:, c, :],
                            OP.add,
                            OP.mult,
                        )
                    v_sb = gates.tile([128, 2, BL], F32, tag="v")
                    nc.vector.tensor_tensor(
                        v_sb[:], u_sb[:], psB[:, c0:c1, :], OP.add
                    )
                    s_sb = gates.tile([128, 2, BL], F32, tag="s")
                    nc.scalar.activation(s_sb[:], v_sb[:], AF.Sigmoid, scale=2.0)

                    # o1 = z*g (off-chain), o2 = (z-1)*s, g_new = o1 - 2*o2
                    o1_sb = gates.tile([128, 2, BL], F32, tag="o1")
                    nc.vector.tensor_tensor(
                        o1_sb[:], zh, hT[:, c0:c1, :].bitcast(F32), OP.mult
                    )
                    o2_sb = gates.tile([128, 2, BL], F32, tag="o2")
                    nc.vector.scalar_tensor_tensor(
                        o2_sb[:], zh, 1.0, s_sb[:], OP.subtract, OP.mult
                    )
                    nc.vector.scalar_tensor_tensor(
                        hT_new[:, c0:c1, :], o2_sb[:], -2.0, o1_sb[:],
                        OP.mult, OP.add,
                    )
                    if dbg and t == 0:
                        sl = (slice(None), slice(c0, c1), slice(None))
                        for nm, tile_ in (("d_u", u_sb), ("d_v", v_sb), ("d_s", s_sb)):
                            nc.sync.dma_start(dbg_outs[nm].ap()[sl], tile_[:])
                if dbg and t == 0:
                    nc.sync.dma_start(dbg_outs["d_r"].ap()[:], r_sb[:])
                    nc.sync.dma_start(dbg_outs["d_z"].ap()[:], z_sb[:])
                    nc.gpsimd.dma_start(dbg_outs["d_g"].ap()[:], hT_new[:].bitcast(F32))

                hT = hT_new

            if skel < 2:
                psA_f = pA.tile([128, 12, BL], F32, tag="A", name="Afin")
                emit_logits(hT, T_steps - 1, psA_f)
            fence = nc.sync.dma_start(hlast[:], hT[:])

            # ------------- final phase: log_softmax -------------
            # Two sweeps so ACT switches table sets exactly twice (exp, ln)
            # instead of per-group.
            rows = 0 if skel >= 1 else T_steps * BL
            lp_flat = logp.ap().rearrange("t b v -> (t b) v")
            ld_flat = logits_dram[:].rearrange("t b v -> (t b) v")
            G = 4  # tiles of [128, G, V]
            ngroups = rows // (128 * G)
            ls_all = fin.tile([128, max(ngroups, 1), G], F32, tag="lsall")
            # pass 1: exp + row-sum per group
            for i in range(ngroups):
                l_sb = fin.tile([128, G, V], F32, tag="fl")
                nc.sync.dma_start(
                    l_sb[:],
                    ld_flat[i * 128 * G : (i + 1) * 128 * G, :].rearrange(
                        "(c p) v -> p c v", p=128
                    ),
                )
                e_sb = fin.tile([128, G, V], F32, tag="fe")
                e_act = nc.scalar.activation(e_sb[:], l_sb[:], AF.Exp)
                tile.add_dep_helper(
                    e_act.ins, fence.ins, True, "exp after recurrence"
                )
                nc.vector.tensor_reduce(
                    ls_all[:, i, :], e_sb[:], mybir.AxisListType.X, OP.add
                )
            # ln over all sums at once (one table switch)
            nc.scalar.activation(
                ls_all[:].rearrange("p a b -> p (a b)"),
                ls_all[:].rearrange("p a b -> p (a b)"),
                AF.Ln,
            )
            # pass 2: subtract per group (re-read logits)
            for i in range(ngroups):
                l_sb = fin.tile([128, G, V], F32, tag="fl")
                nc.sync.dma_start(
                    l_sb[:],
                    ld_flat[i * 128 * G : (i + 1) * 128 * G, :].rearrange(
                        "(c p) v -> p c v", p=128
                    ),
                )
                o_sb = fin.tile([128, G, V], F32, tag="fo")
                for c in range(G):
                    nc.vector.tensor_scalar(
                        o_sb[:, c, :],
                        l_sb[:, c, :],
                        ls_all[:, i, c : c + 1],
                        None,
                        OP.subtract,
                    )
                nc.sync.dma_start(
                    lp_flat[i * 128 * G : (i + 1) * 128 * G, :].rearrange(
                        "(c p) v -> p c v", p=128
                    ),
                    o_sb[:],
                )

    nc.compile()
    return nc


def _prep_host(encoder_hidden, target_tensor, embedding, W_ih, W_hh, b_ih, b_hh,
               W_out, b_out, T_steps=T):
    """Build per-core input maps (host-side data marshalling only)."""
    f32 = np.float32
    # teacher-forced tokens [T, B]
    tok = np.concatenate(
        [np.zeros((B, 1), dtype=target_tensor.dtype), target_tensor[:, : T - 1]],
        axis=1,
    ).T[:T_steps]  # [T_steps, B]

    eye = np.eye(V, dtype=f32)
    whht_np = np.ascontiguousarray(
        W_hh.T.astype(f32).reshape(KC, 128, NC3, 128).transpose(1, 0, 2, 3)
    )
    # state is g = h + 1 on device; W*g = W*h + rowsum(W), so subtract
    # rowsum(W) from each bias term.
    whh_rowsum = W_hh.astype(np.float64).sum(axis=1).astype(f32)  # [3H]
    wih_aug_np = np.zeros((128, 3 * H), dtype=f32)
    wih_aug_np[:EMB] = W_ih.T.astype(f32)
    btot = b_ih.astype(f32).copy()
    btot[: 2 * H] += b_hh[: 2 * H].astype(f32)
    btot[: 2 * H] -= whh_rowsum[: 2 * H]
    wih_aug_np[EMB] = btot
    embt_np = np.ascontiguousarray(embedding.T.astype(f32))
    bhhn_np = np.ascontiguousarray(
        (b_hh[2 * H :].astype(f32) - whh_rowsum[2 * H :]).reshape(KC, 128).T
    )
    woutt_np = np.ascontiguousarray(
        W_out.T.astype(f32).reshape(KC, 128, V).transpose(1, 0, 2)
    )
    wout_rowsum = W_out.astype(np.float64).sum(axis=1).astype(f32)  # [V]
    bout_np = np.ascontiguousarray(
        (b_out.astype(f32) - wout_rowsum).reshape(1, V)
    )

    in_maps = []
    for c in range(NCORES):
        b0 = c * BL
        h0 = encoder_hidden[0, b0 : b0 + BL, :].astype(f32) + 1.0  # g = h+1
        h0t_np = np.ascontiguousarray(
            h0.T.reshape(KC, 128, BL).transpose(1, 0, 2)
        )  # [128, KC, BL]
        oh_np = np.ascontiguousarray(
            eye[tok[:, b0 : b0 + BL]].transpose(0, 2, 1)
        )  # [T, V=128, BL]
        in_maps.append(
            {
                "h0t": h0t_np,
                "onehot": oh_np,
                "whht": whht_np,
                "embt": embt_np,
                "wih_aug": wih_aug_np,
                "bhhn": bhhn_np,
                "woutt": woutt_np,
                "bout": bout_np,
            }
        )
    return in_maps


def kernel(encoder_outputs, encoder_hidden, target_tensor, embedding, W_ih, W_hh,
           b_ih, b_hh, W_out, b_out, _trace=False, _T=T):
    global LAST_EXEC_NS
    encoder_hidden = np.asarray(encoder_hidden)
    target_tensor = np.asarray(target_tensor)
    key = (_T, _trace)
    if key not in _CACHE:
        _CACHE[key] = build(T_steps=_T, trace=_trace)
    nc = _CACHE[key]

    in_maps = _prep_host(
        encoder_hidden, target_tensor, np.asarray(embedding), np.asarray(W_ih),
        np.asarray(W_hh), np.asarray(b_ih), np.asarray(b_hh), np.asarray(W_out),
        np.asarray(b_out), T_steps=_T,
    )
    res = run_bass_kernel_spmd(
        nc, in_maps, core_ids=list(range(NCORES)), trace=_trace
    )
    LAST_EXEC_NS = res.exec_time_ns

    log_probs = np.empty((_T, B, V), dtype=np.float32)
    hidden = np.empty((1, B, H), dtype=np.float32)
    for c in range(NCORES):
        b0 = c * BL
        out = res.results[c]
        log_probs[:, b0 : b0 + BL, :] = out["logp"]
        hl = out["hlast"]  # [128, KC, BL] (g = h+1)
        hidden[0, b0 : b0 + BL, :] = hl.transpose(1, 0, 2).reshape(H, BL).T - 1.0
    return log_probs, hidden
